# revision 38
# baseline (speedup 1.0000x reference)
"""Trainium2 Bass kernel for EntityAwareLSTMLayer.

Problem (hardcoded):
  B=1024, T=365, DYN=32, STATIC=27, UNITS=256
  i_gate = sigmoid(x_static @ W_sh + bias_s)            [B, U]   (static, once)
  gates  = x_t @ W_ih + bias + h @ W_hh                 [B, 3U]  (f|o|g)
  c      = sigmoid(f) * c + i_gate * tanh(g)
  h      = sigmoid(o) * tanh(c)
  return h_final                                        [B, U]

Sharding: data-parallel over batch, 8 cores x 128 rows.

Transposed state: per-step tensors live as X^T with units on partitions
and batch on the free dim. The recurrent matmuls keep the constant W_hh
blocks stationary in the PE and stream h^T, so the h produced by the
DVE feeds the next step's matmuls directly - no per-step transpose.

Two phase-offset chains: the 128-row batch is split into two 64-wide
chains (A: batch 0-63, B: 64-127) whose step recurrences are
independent. Their blocks are emitted alternately, so in steady state
chain B runs half a step behind chain A and the serial
ACT->DVE->ACT->DVE chain of one hides under the other's matmuls,
roughly halving effective latency per step.

Activation packing: sigmoid(x) = 0.5*tanh(x/2)+0.5; the 1/2 is folded
into the f and o columns of W_ih/W_hh/bias at load time, so one ACT op
per chain computes tanh over the packed [f/2 | g | o/2] PSUM block and
the sigmoids finish as fused (x*0.5+0.5) tensor_scalar DVE ops.

Gate PSUM layout per chain (one 2KB bank per generation, fp32):
cols f0 0:64 f1 64:128 g0 128:192 g1 192:256 o0 256:320 o1 320:384.
The first matmul into a generation (an x matmul) uses start=True
(whole-bank has_written clear); later chunks overwrite-then-accumulate
via the per-element bits.

The PE_HAM clock gate holds the PE at 1.2 GHz until it sees a ~3.4us
fully-busy window; a warm-up burst through the DMA preamble plus small
per-step filler matmuls keep it at 2.4 GHz for the whole scan.

x_dynamic is pre-transposed on the host to [128, 92*128] fp16:
partition 32*(t%4)+d, free col (t//4)*128 + batch; loaded in 23
group-DMAs so the scan starts as soon as group 0 lands.
"""

import numpy as np

B_L = 128  # batch rows per core
BC = 64  # batch rows per chain
T = 365
TP = 368  # T padded to a multiple of 4
DYN = 32
STATIC = 27
U = 256
NCORES = 8
NCHUNK = TP // 4  # 92 x chunks (4 timesteps each)
GCH = 4  # chunks per DMA group
NG = NCHUNK // GCH  # 23 groups

# W-column chunk m (128 cols of 3U; f=0,1 o=2,3 g=4,5) -> PSUM col block
BLK = {0: 0, 1: 64, 4: 128, 5: 192, 2: 256, 3: 320}
MORDER = [0, 1, 4, 5, 2, 3]  # f, g, o

_cached = {}


def _build_program(has_bias: bool, fill_n: int = 6):
    from contextlib import ExitStack

    import concourse.bacc as bacc
    import concourse.tile as tile
    from concourse import mybir

    f32 = mybir.dt.float32
    f16 = mybir.dt.float16
    AF = mybir.ActivationFunctionType
    ALU = mybir.AluOpType

    nc = bacc.Bacc("TRN2", target_bir_lowering=False, debug=False)

    x_dyn = nc.dram_tensor(
        "x_dynamic", [B_L, NCHUNK * B_L], f16, kind="ExternalInput"
    )
    x_st = nc.dram_tensor("x_static", [B_L, STATIC], f32, kind="ExternalInput")
    w_ih = nc.dram_tensor("weight_ih", [DYN, 3 * U], f32, kind="ExternalInput")
    w_hh = nc.dram_tensor("weight_hh", [U, 3 * U], f32, kind="ExternalInput")
    w_sh = nc.dram_tensor("weight_sh", [STATIC, U], f32, kind="ExternalInput")
    bias = nc.dram_tensor("bias", [1, 3 * U], f32, kind="ExternalInput")
    bias_s = nc.dram_tensor("bias_s", [1, U], f32, kind="ExternalInput")
    out = nc.dram_tensor("out", [B_L, 2 * B_L], f32, kind="ExternalOutput")

    with tile.TileContext(nc) as tc, ExitStack() as ctx:
        const = ctx.enter_context(tc.tile_pool(name="const", bufs=1))
        fill_ps = ctx.enter_context(
            tc.tile_pool(name="fillp", bufs=1, space="PSUM")
        )
        wsrc = const.tile([128, 256], f16, tag="wsrc")
        nc.vector.memset(wsrc[:], 0.25)
        wps = fill_ps.tile([128, 256], f32, tag="fill")
        # warm-up burst: PE_HAM releases the clock gate (1.2 -> 2.4 GHz) only
        # after a ~3.4us fully-busy window; burn through the DMA preamble.
        for _ in range(24):
            nc.tensor.matmul(wps[:], wsrc[:, 0:128], wsrc[:], start=True, stop=True)

        def pe_filler(n):
            # keep the PE from idling long enough for the HAM to re-throttle
            for _ in range(n):
                nc.tensor.matmul(
                    wps[:], wsrc[:, 0:128], wsrc[:], start=True, stop=True
                )

        # group g holds chunks 4g..4g+3 side by side: [128, 4*B_L]
        xgtiles = [
            const.tile([128, GCH * B_L], f16, tag=f"xg{g}", name=f"xg{g}")
            for g in range(NG)
        ]
        Wih4 = const.tile([128, 3 * U], f16)  # W_ih replicated at 4 bases
        Whh0 = const.tile([128, 3 * U], f16)
        Whh1 = const.tile([128, 3 * U], f16)
        Wshb = const.tile([STATIC + 1, U], f16)  # rows 0-26 W_sh, row 27 bias_s
        xsT = const.tile([128, B_L], f16)
        igc = [
            const.tile([128, 2 * BC], f16, tag=f"ig{ch}", name=f"ig{ch}")
            for ch in range(2)
        ]
        if has_bias:
            ones_row = const.tile([1, B_L], f16)
            bias16 = const.tile([1, 3 * U], f16)

        # one bank per generation per chain; two bufs=1 pools alternated
        # manually so a new generation's writers only depend on the
        # generation before last (pool-level rotation would serialize the
        # next step's x matmuls behind this step's ACT reads)
        pools = [
            [
                ctx.enter_context(
                    tc.tile_pool(name=f"ps{ch}_{par}", bufs=1, space="PSUM")
                )
                for par in range(2)
            ]
            for ch in range(2)
        ]

        st = ctx.enter_context(tc.tile_pool(name="state", bufs=2))
        tmp = ctx.enter_context(tc.tile_pool(name="tmp", bufs=3))

        cT = [
            st.tile([128, 2 * BC], f16, tag=f"c{ch}", name=f"c{ch}")
            for ch in range(2)
        ]
        hT = [
            st.tile([128, 2 * BC], f16, tag=f"h{ch}", name=f"h{ch}")
            for ch in range(2)
        ]
        for ch in range(2):
            nc.vector.memset(cT[ch][:], 0.0)
            nc.vector.memset(hT[ch][:], 0.0)

        with tc.tile_pool(name="stage", bufs=1) as stage:
            # --- weights; fold 1/2 into the f and o columns (0:512) so that
            # sigmoid(f) = 0.5*tanh(f/2)+0.5 shares one tanh with g ---
            wst = stage.tile([128, 3 * U], f32)
            nc.sync.dma_start(wst[:], w_hh[0:128, :])
            nc.vector.tensor_scalar_mul(Whh0[:, 0:512], wst[:, 0:512], 0.25)
            nc.vector.tensor_scalar_mul(Whh0[:, 512:768], wst[:, 512:768], 0.5)
            nc.sync.dma_start(wst[:], w_hh[128:256, :])
            nc.vector.tensor_scalar_mul(Whh1[:, 0:512], wst[:, 0:512], 0.25)
            nc.vector.tensor_scalar_mul(Whh1[:, 512:768], wst[:, 512:768], 0.5)
            wih32 = stage.tile([DYN, 3 * U], f32)
            nc.sync.dma_start(wih32[:], w_ih[:])
            for g in range(4):
                nc.vector.tensor_scalar_mul(
                    Wih4[32 * g : 32 * g + 32, 0:512], wih32[:, 0:512], 0.5
                )
                nc.vector.tensor_copy(
                    Wih4[32 * g : 32 * g + 32, 512:768], wih32[:, 512:768]
                )
            wsh32 = stage.tile([STATIC, U], f32)
            nc.sync.dma_start(wsh32[:], w_sh[:])
            nc.vector.tensor_copy(Wshb[0:STATIC, :], wsh32[:])
            bs32 = stage.tile([1, U], f32)
            nc.sync.dma_start(bs32[:], bias_s[:])
            bs16 = stage.tile([1, U], f16)
            nc.vector.tensor_copy(bs16[:], bs32[:])
            # partition 27 is not engine-addressable; DMA has no such limit
            nc.sync.dma_start(Wshb[STATIC : STATIC + 1, :], bs16[:])
            if has_bias:
                b32 = stage.tile([1, 3 * U], f32)
                nc.sync.dma_start(b32[:], bias[:])
                nc.vector.tensor_scalar_mul(bias16[:, 0:512], b32[:, 0:512], 0.5)
                nc.vector.tensor_copy(bias16[:, 512:768], b32[:, 512:768])
                nc.vector.memset(ones_row[:], 1.0)

            # --- x_static -> transposed [27, 128] + ones row 27 ---
            xst32 = stage.tile([B_L, STATIC], f32)
            nc.sync.dma_start(xst32[:], x_st[:])
            xst16 = stage.tile([B_L, 128], f16)
            nc.vector.memset(xst16[:], 0.0)
            nc.vector.tensor_copy(xst16[:, 0:STATIC], xst32[:])
            # ones in column 27 become the ones row after the transpose
            nc.vector.memset(xst16[:, STATIC : STATIC + 1], 1.0)
            nc.sync.dma_start_transpose(xsT[:], xst16[:])

            # --- igT = sigmoid(x_static @ W_sh + bias_s)^T, then per-chain
            # [u0 bslice | u1 bslice] copies ---
            ig_ps = pools[0][1].tile([128, 512], f32, tag="ps", name="ig_ps")
            for m in range(2):
                nc.tensor.matmul(
                    ig_ps[:, 128 * m : 128 * (m + 1)],
                    Wshb[:, 128 * m : 128 * (m + 1)],
                    xsT[0 : STATIC + 1, :],
                    start=(m == 0),
                    stop=(m == 1),
                )
            igT = stage.tile([128, 2 * B_L], f16)
            nc.scalar.activation(igT[:], ig_ps[:, 0 : 2 * B_L], AF.Sigmoid)
            for ch in range(2):
                for k in range(2):
                    nc.vector.tensor_copy(
                        igc[ch][:, BC * k : BC * (k + 1)],
                        igT[:, 128 * k + BC * ch : 128 * k + BC * ch + BC],
                    )

            # --- x_dynamic: host-transposed; group 0 first for a fast start ---
            for g in range(NG):
                nc.sync.dma_start(
                    xgtiles[g][:], x_dyn[:, g * GCH * B_L : (g + 1) * GCH * B_L]
                )

        def x_matmuls(t, ch, ps):
            # gT[m] = W_ih[:, m-block]^T @ x_t^T ; K=32 row-group at 32*(t%4)
            g4 = 32 * (t % 4)
            cc = t // 4
            xt = xgtiles[cc // GCH][
                g4 : g4 + 32,
                (cc % GCH) * B_L + BC * ch : (cc % GCH) * B_L + BC * ch + BC,
            ]
            for i, m in enumerate(MORDER):
                nc.tensor.matmul(
                    ps[:, BLK[m] : BLK[m] + BC],
                    Wih4[g4 : g4 + 32, 128 * m : 128 * (m + 1)],
                    xt,
                    start=(i == 0),
                    stop=False,
                    tile_position=(g4, 0),
                )
                if has_bias:
                    nc.tensor.matmul(
                        ps[:, BLK[m] : BLK[m] + BC],
                        bias16[0:1, 128 * m : 128 * (m + 1)],
                        ones_row[0:1, 0:BC],
                        start=False,
                        stop=False,
                    )

        ps_cur = [None, None]
        for ch in range(2):
            ps_cur[ch] = pools[ch][0].tile(
                [128, 512], f32, tag="ps", name=f"ps{ch}_init"
            )
            x_matmuls(0, ch, ps_cur[ch])

        for t in range(T):
            last = t == T - 1
            for ch in range(2):
                ps = ps_cur[ch]
                mms = []
                for ms, ks in (
                    ((0, 1, 4, 5), (0,)),
                    ((0, 1, 4, 5), (1,)),
                    ((2, 3), (0,)),
                    ((2, 3), (1,)),
                ):
                    for m in ms:
                        for k in ks:
                            Whh = Whh0 if k == 0 else Whh1
                            mms.append(
                                nc.tensor.matmul(
                                    ps[:, BLK[m] : BLK[m] + BC],
                                    Whh[:, 128 * m : 128 * (m + 1)],
                                    hT[ch][:, BC * k : BC * (k + 1)],
                                    start=False,
                                    stop=(m == 3 and k == 1),
                                )
                            )
                for a, b in zip(mms[1:], mms[:-1]):
                    tile.add_dep_helper(
                        a.ins, b.ins, sync=False, reason="keep MM order"
                    )
                if not last:
                    ps_cur[ch] = pools[ch][(t + 1) % 2].tile(
                        [128, 512], f32, tag="ps", name=f"ps{ch}_{t}"
                    )
                    x_matmuls(t + 1, ch, ps_cur[ch])
                pe_filler(fill_n)

                # tanh over packed [f/2 | g]; o/2 separately (off-chain)
                tgt = tmp.tile([128, 256], f16, tag=f"tgt{ch}")
                nc.scalar.activation(tgt[:], ps[:, 0:256], AF.Tanh)
                to = tmp.tile([128, 2 * BC], f16, tag=f"to{ch}")
                nc.scalar.activation(to[:], ps[:, 256:384], AF.Tanh)
                # to1 = to+1 precomputed off-chain so H = to1*tc is a fast TT
                to1 = tmp.tile([128, 2 * BC], f16, tag=f"to1{ch}")
                nc.vector.tensor_scalar_add(to1[:], to[:], 1.0)
                # m2 on GPSIMD in parallel with u on DVE
                m2 = tmp.tile([128, 2 * BC], f16, tag=f"m2{ch}")
                nc.gpsimd.tensor_mul(m2[:], igc[ch][:], tgt[:, 128:256])
                # u = (tf+1)*c = 2*sigmoid(f)*c ; c' = u*0.5 + ig*tg
                u = tmp.tile([128, 2 * BC], f16, tag=f"u{ch}")
                nc.vector.scalar_tensor_tensor(
                    u[:], tgt[:, 0:128], 1.0, cT[ch][:], ALU.add, ALU.mult
                )
                c_new = st.tile([128, 2 * BC], f16, tag=f"c{ch}", name=f"cn{ch}")
                nc.vector.scalar_tensor_tensor(
                    c_new[:], u[:], 0.5, m2[:], ALU.mult, ALU.add
                )

                if last:
                    tch = tmp.tile([128, 2 * BC], f32, tag=f"tc32{ch}")
                    nc.scalar.activation(tch[:], c_new[:], AF.Tanh)
                    # H = (to+1)*tanh(c) = 2h ; host halves the output
                    h_out = tmp.tile([128, 2 * BC], f32, tag=f"hout{ch}")
                    nc.vector.tensor_mul(h_out[:], to1[:], tch[:])
                    nc.sync.dma_start(
                        out[:, 128 * ch : 128 * (ch + 1)], h_out[:]
                    )
                else:
                    tc16 = tmp.tile([128, 2 * BC], f16, tag=f"tc{ch}")
                    nc.scalar.activation(tc16[:], c_new[:], AF.Tanh)
                    # H = (to+1)*tanh(c) = 2h; W_hh columns carry the 1/2
                    h_new = st.tile([128, 2 * BC], f16, tag=f"h{ch}", name=f"hn{ch}")
                    nc.vector.tensor_mul(h_new[:], to1[:], tc16[:])
                    hT[ch] = h_new
                cT[ch] = c_new

    nc.compile()
    return nc


def get_program(has_bias: bool = False):
    if has_bias not in _cached:
        _cached[has_bias] = _build_program(has_bias)
    return _cached[has_bias]


def _host_transpose_x(x_core: np.ndarray) -> np.ndarray:
    """[B_L, T, DYN] fp32 -> [128, NCHUNK*B_L] fp16 on-chip layout:
    row 32*k+d (k = t%4), col (t//4)*B_L + b."""
    src = np.zeros((TP, DYN, B_L), dtype=np.float16)
    src[:T] = x_core.transpose(1, 2, 0)  # [T, DYN, B_L]
    return np.ascontiguousarray(
        src.reshape(NCHUNK, 4, DYN, B_L)
        .transpose(1, 2, 0, 3)
        .reshape(4 * DYN, NCHUNK * B_L)
    )


def make_in_maps(inputs):
    x_dynamic = np.asarray(inputs["x_dynamic"], dtype=np.float32)
    x_static = np.asarray(inputs["x_static"], dtype=np.float32)
    w_ih = np.ascontiguousarray(np.asarray(inputs["weight_ih"], dtype=np.float32))
    w_hh = np.ascontiguousarray(np.asarray(inputs["weight_hh"], dtype=np.float32))
    w_sh = np.ascontiguousarray(np.asarray(inputs["weight_sh"], dtype=np.float32))
    bias = np.ascontiguousarray(
        np.asarray(inputs["bias"], dtype=np.float32).reshape(1, 3 * U)
    )
    bias_s = np.ascontiguousarray(
        np.asarray(inputs["bias_s"], dtype=np.float32).reshape(1, U)
    )
    in_maps = []
    for i in range(NCORES):
        sl = slice(i * B_L, (i + 1) * B_L)
        in_maps.append(
            {
                "x_dynamic": _host_transpose_x(x_dynamic[sl]),
                "x_static": np.ascontiguousarray(x_static[sl]),
                "weight_ih": w_ih,
                "weight_hh": w_hh,
                "weight_sh": w_sh,
                "bias": bias,
                "bias_s": bias_s,
            }
        )
    return in_maps


def _untranspose_out(o: np.ndarray) -> np.ndarray:
    """[128, 256] (out[p, 128*ch + 64*k + b] = h[64*ch+b, 128*k+p]) -> [B_L, U]"""
    return np.ascontiguousarray(
        (0.5 * o).reshape(128, 2, 2, BC).transpose(1, 3, 2, 0).reshape(B_L, U)
    )


def kernel(**inputs) -> np.ndarray:
    from concourse.bass_utils import run_bass_kernel_spmd

    has_bias = bool(np.any(np.asarray(inputs["bias"])))
    nc = get_program(has_bias)
    in_maps = make_in_maps(inputs)
    res = run_bass_kernel_spmd(nc, in_maps, core_ids=list(range(NCORES)))
    return np.concatenate(
        [_untranspose_out(r["out"]) for r in res.results], axis=0
    ).astype(np.float32)


# revision 39
# speedup vs baseline: 1.2058x; 1.2058x over previous
"""Trainium2 Bass kernel for EntityAwareLSTMLayer.

Problem (hardcoded):
  B=1024, T=365, DYN=32, STATIC=27, UNITS=256
  i_gate = sigmoid(x_static @ W_sh + bias_s)            [B, U]   (static, once)
  gates  = x_t @ W_ih + bias + h @ W_hh                 [B, 3U]  (f|o|g)
  c      = sigmoid(f) * c + i_gate * tanh(g)
  h      = sigmoid(o) * tanh(c)
  return h_final                                        [B, U]

Sharding: data-parallel over batch, 8 cores x 128 rows.

Transposed state: per-step tensors live as X^T with units on partitions
and batch on the free dim. The recurrent matmuls keep the constant W_hh
blocks stationary in the PE and stream h^T, so the h produced by the
DVE feeds the next step's matmuls directly - no per-step transpose.

Two phase-offset chains: the 128-row batch is split into two 64-wide
chains (A: batch 0-63, B: 64-127) whose step recurrences are
independent. Their blocks are emitted alternately, so in steady state
chain B runs half a step behind chain A and the serial
ACT->DVE->ACT->DVE chain of one hides under the other's matmuls,
roughly halving effective latency per step.

Activation packing: sigmoid(x) = 0.5*tanh(x/2)+0.5; the 1/2 is folded
into the f and o columns of W_ih/W_hh/bias at load time, so one ACT op
per chain computes tanh over the packed [f/2 | g | o/2] PSUM block and
the sigmoids finish as fused (x*0.5+0.5) tensor_scalar DVE ops.

Gate PSUM layout per chain (one 2KB bank per generation, fp32):
cols f0 0:64 f1 64:128 g0 128:192 g1 192:256 o0 256:320 o1 320:384.
The first matmul into a generation (an x matmul) uses start=True
(whole-bank has_written clear); later chunks overwrite-then-accumulate
via the per-element bits.

The PE_HAM clock gate holds the PE at 1.2 GHz until it sees a ~3.4us
fully-busy window; a warm-up burst through the DMA preamble plus small
per-step filler matmuls keep it at 2.4 GHz for the whole scan.

x_dynamic is pre-transposed on the host to [128, 92*128] fp16:
partition 32*(t%4)+d, free col (t//4)*128 + batch; loaded in 23
group-DMAs so the scan starts as soon as group 0 lands.
"""

import numpy as np

B_L = 128  # batch rows per core
BC = 64  # batch rows per chain
T = 365
TP = 368  # T padded to a multiple of 4
DYN = 32
STATIC = 27
U = 256
NCORES = 8
NCHUNK = TP // 4  # 92 x chunks (4 timesteps each)
GCH = 4  # chunks per DMA group
NG = NCHUNK // GCH  # 23 groups

# W-column chunk m (128 cols of 3U; f=0,1 o=2,3 g=4,5) -> PSUM col block
BLK = {0: 0, 1: 64, 4: 128, 5: 192, 2: 256, 3: 320}
MORDER = [0, 1, 4, 5, 2, 3]  # f, g, o

_cached = {}


def _build_program(has_bias: bool, fill_n: int = 6):
    from contextlib import ExitStack

    import concourse.bacc as bacc
    import concourse.tile as tile
    from concourse import mybir

    f32 = mybir.dt.float32
    f16 = mybir.dt.float16
    AF = mybir.ActivationFunctionType
    ALU = mybir.AluOpType

    nc = bacc.Bacc("TRN2", target_bir_lowering=False, debug=False)

    x_dyn = nc.dram_tensor(
        "x_dynamic", [B_L, NCHUNK * B_L], f16, kind="ExternalInput"
    )
    x_st = nc.dram_tensor("x_static", [B_L, STATIC], f32, kind="ExternalInput")
    w_ih = nc.dram_tensor("weight_ih", [DYN, 3 * U], f32, kind="ExternalInput")
    w_hh = nc.dram_tensor("weight_hh", [U, 3 * U], f32, kind="ExternalInput")
    w_sh = nc.dram_tensor("weight_sh", [STATIC, U], f32, kind="ExternalInput")
    bias = nc.dram_tensor("bias", [1, 3 * U], f32, kind="ExternalInput")
    bias_s = nc.dram_tensor("bias_s", [1, U], f32, kind="ExternalInput")
    out = nc.dram_tensor("out", [B_L, 2 * B_L], f32, kind="ExternalOutput")

    with tile.TileContext(nc) as tc, ExitStack() as ctx:
        const = ctx.enter_context(tc.tile_pool(name="const", bufs=1))
        fill_ps = ctx.enter_context(
            tc.tile_pool(name="fillp", bufs=1, space="PSUM")
        )
        wsrc = const.tile([128, 256], f16, tag="wsrc")
        nc.vector.memset(wsrc[:], 0.25)
        wps = fill_ps.tile([128, 256], f32, tag="fill")
        # warm-up burst: PE_HAM releases the clock gate (1.2 -> 2.4 GHz) only
        # after a ~3.4us fully-busy window; burn through the DMA preamble.
        for _ in range(24):
            nc.tensor.matmul(wps[:], wsrc[:, 0:128], wsrc[:], start=True, stop=True)

        def pe_filler(n):
            # keep the PE from idling long enough for the HAM to re-throttle
            for _ in range(n):
                nc.tensor.matmul(
                    wps[:], wsrc[:, 0:128], wsrc[:], start=True, stop=True
                )

        # group g holds chunks 4g..4g+3 side by side: [128, 4*B_L]
        xgtiles = [
            const.tile([128, GCH * B_L], f16, tag=f"xg{g}", name=f"xg{g}")
            for g in range(NG)
        ]
        Wih4 = const.tile([128, 3 * U], f16)  # W_ih replicated at 4 bases
        Whh0 = const.tile([128, 3 * U], f16)
        Whh1 = const.tile([128, 3 * U], f16)
        Wshb = const.tile([STATIC + 1, U], f16)  # rows 0-26 W_sh, row 27 bias_s
        xsT = const.tile([128, B_L], f16)
        igc = [
            const.tile([128, 2 * BC], f16, tag=f"ig{ch}", name=f"ig{ch}")
            for ch in range(2)
        ]
        if has_bias:
            ones_row = const.tile([1, B_L], f16)
            bias16 = const.tile([1, 3 * U], f16)

        # one bank per generation per chain; two bufs=1 pools alternated
        # manually so a new generation's writers only depend on the
        # generation before last (pool-level rotation would serialize the
        # next step's x matmuls behind this step's ACT reads)
        pools = [
            [
                ctx.enter_context(
                    tc.tile_pool(name=f"ps{ch}_{par}", bufs=1, space="PSUM")
                )
                for par in range(2)
            ]
            for ch in range(2)
        ]

        st = ctx.enter_context(tc.tile_pool(name="state", bufs=2))
        tmp = ctx.enter_context(tc.tile_pool(name="tmp", bufs=3))

        cT = [
            st.tile([128, 2 * BC], f16, tag=f"c{ch}", name=f"c{ch}")
            for ch in range(2)
        ]
        hT = [
            st.tile([128, 2 * BC], f16, tag=f"h{ch}", name=f"h{ch}")
            for ch in range(2)
        ]
        for ch in range(2):
            nc.vector.memset(cT[ch][:], 0.0)
            nc.vector.memset(hT[ch][:], 0.0)

        with tc.tile_pool(name="stage", bufs=1) as stage:
            # --- weights; fold 1/2 into the f and o columns (0:512) so that
            # sigmoid(f) = 0.5*tanh(f/2)+0.5 shares one tanh with g ---
            wst = stage.tile([128, 3 * U], f32)
            nc.sync.dma_start(wst[:], w_hh[0:128, :])
            nc.vector.tensor_scalar_mul(Whh0[:, 0:512], wst[:, 0:512], 0.25)
            nc.vector.tensor_scalar_mul(Whh0[:, 512:768], wst[:, 512:768], 0.5)
            nc.sync.dma_start(wst[:], w_hh[128:256, :])
            nc.vector.tensor_scalar_mul(Whh1[:, 0:512], wst[:, 0:512], 0.25)
            nc.vector.tensor_scalar_mul(Whh1[:, 512:768], wst[:, 512:768], 0.5)
            wih32 = stage.tile([DYN, 3 * U], f32)
            nc.sync.dma_start(wih32[:], w_ih[:])
            for g in range(4):
                nc.vector.tensor_scalar_mul(
                    Wih4[32 * g : 32 * g + 32, 0:512], wih32[:, 0:512], 0.5
                )
                nc.vector.tensor_copy(
                    Wih4[32 * g : 32 * g + 32, 512:768], wih32[:, 512:768]
                )
            wsh32 = stage.tile([STATIC, U], f32)
            nc.sync.dma_start(wsh32[:], w_sh[:])
            nc.vector.tensor_copy(Wshb[0:STATIC, :], wsh32[:])
            bs32 = stage.tile([1, U], f32)
            nc.sync.dma_start(bs32[:], bias_s[:])
            bs16 = stage.tile([1, U], f16)
            nc.vector.tensor_copy(bs16[:], bs32[:])
            # partition 27 is not engine-addressable; DMA has no such limit
            nc.sync.dma_start(Wshb[STATIC : STATIC + 1, :], bs16[:])
            if has_bias:
                b32 = stage.tile([1, 3 * U], f32)
                nc.sync.dma_start(b32[:], bias[:])
                nc.vector.tensor_scalar_mul(bias16[:, 0:512], b32[:, 0:512], 0.5)
                nc.vector.tensor_copy(bias16[:, 512:768], b32[:, 512:768])
                nc.vector.memset(ones_row[:], 1.0)

            # --- x_static -> transposed [27, 128] + ones row 27 ---
            xst32 = stage.tile([B_L, STATIC], f32)
            nc.sync.dma_start(xst32[:], x_st[:])
            xst16 = stage.tile([B_L, 128], f16)
            nc.vector.memset(xst16[:], 0.0)
            nc.vector.tensor_copy(xst16[:, 0:STATIC], xst32[:])
            # ones in column 27 become the ones row after the transpose
            nc.vector.memset(xst16[:, STATIC : STATIC + 1], 1.0)
            nc.sync.dma_start_transpose(xsT[:], xst16[:])

            # --- igT = sigmoid(x_static @ W_sh + bias_s)^T, then per-chain
            # [u0 bslice | u1 bslice] copies ---
            ig_ps = pools[0][1].tile([128, 512], f32, tag="ps", name="ig_ps")
            for m in range(2):
                nc.tensor.matmul(
                    ig_ps[:, 128 * m : 128 * (m + 1)],
                    Wshb[:, 128 * m : 128 * (m + 1)],
                    xsT[0 : STATIC + 1, :],
                    start=(m == 0),
                    stop=(m == 1),
                )
            igT = stage.tile([128, 2 * B_L], f16)
            nc.scalar.activation(igT[:], ig_ps[:, 0 : 2 * B_L], AF.Sigmoid)
            for ch in range(2):
                for k in range(2):
                    nc.vector.tensor_copy(
                        igc[ch][:, BC * k : BC * (k + 1)],
                        igT[:, 128 * k + BC * ch : 128 * k + BC * ch + BC],
                    )

            # --- x_dynamic: host-transposed; group 0 first for a fast start ---
            for g in range(NG):
                nc.sync.dma_start(
                    xgtiles[g][:], x_dyn[:, g * GCH * B_L : (g + 1) * GCH * B_L]
                )

        def x_matmuls(t, ch, ps):
            # gT[m] = W_ih[:, m-block]^T @ x_t^T ; K=32 row-group at 32*(t%4)
            g4 = 32 * (t % 4)
            cc = t // 4
            xt = xgtiles[cc // GCH][
                g4 : g4 + 32,
                (cc % GCH) * B_L + BC * ch : (cc % GCH) * B_L + BC * ch + BC,
            ]
            for i, m in enumerate(MORDER):
                nc.tensor.matmul(
                    ps[:, BLK[m] : BLK[m] + BC],
                    Wih4[g4 : g4 + 32, 128 * m : 128 * (m + 1)],
                    xt,
                    start=(i == 0),
                    stop=False,
                    tile_position=(g4, 0),
                )
                if has_bias:
                    nc.tensor.matmul(
                        ps[:, BLK[m] : BLK[m] + BC],
                        bias16[0:1, 128 * m : 128 * (m + 1)],
                        ones_row[0:1, 0:BC],
                        start=False,
                        stop=False,
                    )

        ps_cur = [None, None]
        for ch in range(2):
            ps_cur[ch] = pools[ch][0].tile(
                [128, 512], f32, tag="ps", name=f"ps{ch}_init"
            )
            x_matmuls(0, ch, ps_cur[ch])

        for t in range(T):
            last = t == T - 1
            for ch in range(2):
                ps = ps_cur[ch]
                mms = []
                for ms, ks in (
                    ((0, 1, 4, 5), (0,)),
                    ((0, 1, 4, 5), (1,)),
                    ((2, 3), (0,)),
                    ((2, 3), (1,)),
                ):
                    for m in ms:
                        for k in ks:
                            Whh = Whh0 if k == 0 else Whh1
                            mms.append(
                                nc.tensor.matmul(
                                    ps[:, BLK[m] : BLK[m] + BC],
                                    Whh[:, 128 * m : 128 * (m + 1)],
                                    hT[ch][:, BC * k : BC * (k + 1)],
                                    start=False,
                                    stop=(m == 3 and k == 1),
                                )
                            )
                for a, b in zip(mms[1:], mms[:-1]):
                    tile.add_dep_helper(
                        a.ins, b.ins, sync=False, reason="keep MM order"
                    )
                if not last:
                    ps_cur[ch] = pools[ch][(t + 1) % 2].tile(
                        [128, 512], f32, tag="ps", name=f"ps{ch}_{t}"
                    )
                    x_matmuls(t + 1, ch, ps_cur[ch])
                pe_filler(fill_n)

                # tanh over packed [f/2 | g]; o/2 separately (off-chain)
                tgt = tmp.tile([128, 256], f16, tag=f"tgt{ch}")
                nc.scalar.activation(tgt[:], ps[:, 0:256], AF.Tanh)
                to = tmp.tile([128, 2 * BC], f16, tag=f"to{ch}")
                nc.scalar.activation(to[:], ps[:, 256:384], AF.Tanh)
                # to1 = to+1 precomputed off-chain so H = to1*tc is a fast TT
                to1 = tmp.tile([128, 2 * BC], f16, tag=f"to1{ch}")
                nc.vector.tensor_scalar_add(to1[:], to[:], 1.0)
                # m2 first: c_new then follows u with m2 already retired
                m2 = tmp.tile([128, 2 * BC], f16, tag=f"m2{ch}")
                nc.vector.tensor_mul(m2[:], igc[ch][:], tgt[:, 128:256])
                # u = (tf+1)*c = 2*sigmoid(f)*c ; c' = u*0.5 + ig*tg
                u = tmp.tile([128, 2 * BC], f16, tag=f"u{ch}")
                nc.vector.scalar_tensor_tensor(
                    u[:], tgt[:, 0:128], 1.0, cT[ch][:], ALU.add, ALU.mult
                )
                c_new = st.tile([128, 2 * BC], f16, tag=f"c{ch}", name=f"cn{ch}")
                nc.vector.scalar_tensor_tensor(
                    c_new[:], u[:], 0.5, m2[:], ALU.mult, ALU.add
                )

                if last:
                    tch = tmp.tile([128, 2 * BC], f32, tag=f"tc32{ch}")
                    nc.scalar.activation(tch[:], c_new[:], AF.Tanh)
                    # H = (to+1)*tanh(c) = 2h ; host halves the output
                    h_out = tmp.tile([128, 2 * BC], f32, tag=f"hout{ch}")
                    nc.vector.tensor_mul(h_out[:], to1[:], tch[:])
                    nc.sync.dma_start(
                        out[:, 128 * ch : 128 * (ch + 1)], h_out[:]
                    )
                else:
                    tc16 = tmp.tile([128, 2 * BC], f16, tag=f"tc{ch}")
                    nc.scalar.activation(tc16[:], c_new[:], AF.Tanh)
                    # H = (to+1)*tanh(c) = 2h; W_hh columns carry the 1/2
                    h_new = st.tile([128, 2 * BC], f16, tag=f"h{ch}", name=f"hn{ch}")
                    nc.vector.tensor_mul(h_new[:], to1[:], tc16[:])
                    hT[ch] = h_new
                cT[ch] = c_new

    nc.compile()
    return nc


def get_program(has_bias: bool = False):
    if has_bias not in _cached:
        _cached[has_bias] = _build_program(has_bias)
    return _cached[has_bias]


def _host_transpose_x(x_core: np.ndarray) -> np.ndarray:
    """[B_L, T, DYN] fp32 -> [128, NCHUNK*B_L] fp16 on-chip layout:
    row 32*k+d (k = t%4), col (t//4)*B_L + b."""
    src = np.zeros((TP, DYN, B_L), dtype=np.float16)
    src[:T] = x_core.transpose(1, 2, 0)  # [T, DYN, B_L]
    return np.ascontiguousarray(
        src.reshape(NCHUNK, 4, DYN, B_L)
        .transpose(1, 2, 0, 3)
        .reshape(4 * DYN, NCHUNK * B_L)
    )


def make_in_maps(inputs):
    x_dynamic = np.asarray(inputs["x_dynamic"], dtype=np.float32)
    x_static = np.asarray(inputs["x_static"], dtype=np.float32)
    w_ih = np.ascontiguousarray(np.asarray(inputs["weight_ih"], dtype=np.float32))
    w_hh = np.ascontiguousarray(np.asarray(inputs["weight_hh"], dtype=np.float32))
    w_sh = np.ascontiguousarray(np.asarray(inputs["weight_sh"], dtype=np.float32))
    bias = np.ascontiguousarray(
        np.asarray(inputs["bias"], dtype=np.float32).reshape(1, 3 * U)
    )
    bias_s = np.ascontiguousarray(
        np.asarray(inputs["bias_s"], dtype=np.float32).reshape(1, U)
    )
    in_maps = []
    for i in range(NCORES):
        sl = slice(i * B_L, (i + 1) * B_L)
        in_maps.append(
            {
                "x_dynamic": _host_transpose_x(x_dynamic[sl]),
                "x_static": np.ascontiguousarray(x_static[sl]),
                "weight_ih": w_ih,
                "weight_hh": w_hh,
                "weight_sh": w_sh,
                "bias": bias,
                "bias_s": bias_s,
            }
        )
    return in_maps


def _untranspose_out(o: np.ndarray) -> np.ndarray:
    """[128, 256] (out[p, 128*ch + 64*k + b] = h[64*ch+b, 128*k+p]) -> [B_L, U]"""
    return np.ascontiguousarray(
        (0.5 * o).reshape(128, 2, 2, BC).transpose(1, 3, 2, 0).reshape(B_L, U)
    )


def kernel(**inputs) -> np.ndarray:
    from concourse.bass_utils import run_bass_kernel_spmd

    has_bias = bool(np.any(np.asarray(inputs["bias"])))
    nc = get_program(has_bias)
    in_maps = make_in_maps(inputs)
    res = run_bass_kernel_spmd(nc, in_maps, core_ids=list(range(NCORES)))
    return np.concatenate(
        [_untranspose_out(r["out"]) for r in res.results], axis=0
    ).astype(np.float32)
